# revision 6
# baseline (speedup 1.0000x reference)
"""Trainium2 Bass kernel for NeighborsValuesAssigner (retrieval_knn).

out[b,:,h,w] = mean_{n in top8} values[n]  where top8 = 8 largest
score[b,n,h,w] = <p_n, x_patch(b,h,w)> - 0.5||p_n||^2  (5x5 'same' conv).

8 cores, data-parallel over batch (4 images/core). Per core, per
128-pixel tile:
  score  score[px, n] on PE as 3 accumulating fp16 matmuls (hi/lo fp16
         split: xh@ph + xh@pl + xl@ph; error ~2^-22, below fp32
         rounding) -> PSUM [128,1024] halves.
  drain  ACT copies each PSUM half to SBUF f32 (bit-exact); DVE max8
         per half + merge -> t8 = 8th-largest score per pixel.
  mask   DVE is_ge(score_sbuf, t8) -> {0,1} fp16 mask [px, n] (exact:
         same-arithmetic inclusive compare).
Per 512-pixel group (4 tiles), software-pipelined one group behind:
  maskT  PE transposes (fp16; these run at full 2.4 GHz while matmuls
         are power-throttled to 1.2 GHz on this part) -> PSUM -> ACT
         drain to SBUF.
  matmul out[D,px] += (values/8)[n,D]^T @ maskT[n,px] over 16 n-chunks
         (fp16 operands, f32 PSUM accumulation); DVE copy -> DMA out.

Engine budget per 512-px unit: PE ~31us (12 mms/tile + 64 transposes +
16 value mms), ACT ~17us (PSUM drains), DVE ~20us (max8/is_ge/out).
"""
import sys

sys.path.insert(0, "/opt/trn_rl_repo")

import numpy as np

B, C, H, W = 32, 3, 64, 64
N, D = 2048, 128
KH = KW = 5
KDIM = C * KH * KW          # 75
KROWS = KDIM + 1            # 76 = patch dims + bias row
NCORES = 8
BLOC = B // NCORES          # 4 images per core
PX = BLOC * H * W           # 16384 pixels per core
IPX = H * W                 # 4096 pixels per image
NTILE = PX // 128           # 128 pixel-tiles per core
GPX = 512                   # pixels per group
NGRP = PX // GPX            # 32 groups per core
NCHUNK = N // 128           # 16 patch chunks

_CACHE = {}


def _build_program(loop_r=0):
    """loop_r=0: straight-line. loop_r>0: wrap body in a device-side
    For_i loop running it loop_r times (for HW timing via wall deltas)."""
    import concourse.bacc as bacc
    import concourse.tile as tile
    import concourse.mybir as mybir
    from contextlib import ExitStack

    f32 = mybir.dt.float32
    f16 = mybir.dt.float16
    nc = bacc.Bacc("TRN2", target_bir_lowering=False, debug=False)

    xph = nc.dram_tensor("xph", [KROWS, PX], f16, kind="ExternalInput").ap()
    xpl = nc.dram_tensor("xpl", [KROWS, PX], f16, kind="ExternalInput").ap()
    phd = nc.dram_tensor("ph", [KROWS, N], f16, kind="ExternalInput").ap()
    pld = nc.dram_tensor("pl", [KROWS, N], f16, kind="ExternalInput").ap()
    vsd = nc.dram_tensor("vs16", [128, N], f16, kind="ExternalInput").ap()
    idd = nc.dram_tensor("id16", [128, 128], f16, kind="ExternalInput").ap()
    out = nc.dram_tensor("out", [BLOC, 128, H * W], f32, kind="ExternalOutput").ap()

    isge = mybir.AluOpType.is_ge

    with tile.TileContext(nc) as tc, ExitStack() as ctx:
        const = ctx.enter_context(tc.tile_pool(name="const", bufs=1))
        xbig = ctx.enter_context(tc.tile_pool(name="xbig", bufs=1))
        scp = ctx.enter_context(tc.tile_pool(name="scp", bufs=2))
        m16p = ctx.enter_context(tc.tile_pool(name="m16p", bufs=2))
        m8p = ctx.enter_context(tc.tile_pool(name="m8p", bufs=2))
        mkp = ctx.enter_context(tc.tile_pool(name="mkp", bufs=8))
        mtp = ctx.enter_context(tc.tile_pool(name="mtp", bufs=4))
        otp = ctx.enter_context(tc.tile_pool(name="otp", bufs=2))
        psA = ctx.enter_context(tc.tile_pool(name="psA", bufs=2, space="PSUM"))
        pst = ctx.enter_context(tc.tile_pool(name="pst", bufs=2, space="PSUM"))
        psB = ctx.enter_context(tc.tile_pool(name="psB", bufs=2, space="PSUM"))

        ph_t = const.tile([KROWS, N], f16)
        pl_t = const.tile([KROWS, N], f16)
        vs_t = const.tile([128, N], f16)
        id_t = const.tile([128, 128], f16)
        nc.sync.dma_start(ph_t[:], phd[:])
        nc.sync.dma_start(pl_t[:], pld[:])
        nc.sync.dma_start(vs_t[:], vsd[:])
        nc.sync.dma_start(id_t[:], idd[:])

        xh_im = [xbig.tile([KROWS, IPX], f16, tag=f"xh{j}", name=f"xh{j}")
                 for j in range(BLOC)]
        xl_im = [xbig.tile([KROWS, IPX], f16, tag=f"xl{j}", name=f"xl{j}")
                 for j in range(BLOC)]

        loop_cm = tc.For_i(0, loop_r, 1) if loop_r else None
        if loop_cm is not None:
            loop_cm.__enter__()

        for j in range(BLOC):
            nc.sync.dma_start(xh_im[j][:], xph[:, j * IPX:(j + 1) * IPX])
            nc.sync.dma_start(xl_im[j][:], xpl[:, j * IPX:(j + 1) * IPX])

        group_masks = {}

        def phase_a_tiles(g):
            """Score + threshold + mask for the 4 pixel-tiles of group g."""
            masks = []
            for tt in range(4):
                t = 4 * g + tt
                j, toff = divmod(t, IPX // 128)
                tsl = slice(toff * 128, (toff + 1) * 128)
                lh = xh_im[j][:, tsl]
                ll = xl_im[j][:, tsl]
                sc = scp.tile([128, N], f32, tag="sc")
                m16 = m16p.tile([128, 16], f32, tag="m16")
                m8 = m8p.tile([128, 8], f32, tag="m8")
                mk = mkp.tile([128, N], f16, tag="mk")
                for h in range(2):
                    pa = psA.tile([128, 1024], f32, tag="pa")
                    for q in range(2):
                        nsl = slice(h * 1024 + q * 512, h * 1024 + (q + 1) * 512)
                        osl = slice(q * 512, (q + 1) * 512)
                        nc.tensor.matmul(pa[:, osl], lh, ph_t[:, nsl],
                                         start=True, stop=False)
                    for q in range(2):
                        nsl = slice(h * 1024 + q * 512, h * 1024 + (q + 1) * 512)
                        osl = slice(q * 512, (q + 1) * 512)
                        nc.tensor.matmul(pa[:, osl], lh, pl_t[:, nsl],
                                         start=False, stop=False)
                    for q in range(2):
                        nsl = slice(h * 1024 + q * 512, h * 1024 + (q + 1) * 512)
                        osl = slice(q * 512, (q + 1) * 512)
                        nc.tensor.matmul(pa[:, osl], ll, ph_t[:, nsl],
                                         start=False, stop=True)
                    nc.scalar.copy(sc[:, h * 1024:(h + 1) * 1024], pa[:])
                    nc.vector.max(m16[:, h * 8:(h + 1) * 8], pa[:])
                nc.vector.max(m8[:], m16[:])
                for h in range(2):
                    nc.vector.tensor_scalar(
                        mk[:, h * 1024:(h + 1) * 1024],
                        sc[:, h * 1024:(h + 1) * 1024],
                        m8[:, 7:8], None, isge)
                masks.append(mk)
            group_masks[g] = masks

        def phase_b_group(g):
            """Transpose masks + values matmul for group g."""
            b, s = divmod(g, IPX // GPX)
            masks = group_masks.pop(g)
            po = psB.tile([128, GPX], f32, tag="po")
            vq = []
            for c in range(NCHUNK):
                csl = slice(c * 128, (c + 1) * 128)
                pt = pst.tile([128, GPX], f16, tag="pt")
                for tt in range(4):
                    nc.tensor.transpose(pt[:, tt * 128:(tt + 1) * 128],
                                        masks[tt][:, csl], id_t[:])
                if len(vq) >= 2:
                    vq.pop(0)()
                mt = mtp.tile([128, GPX], f16, tag="mt")
                nc.scalar.copy(mt[:], pt[:])

                def vmm(c=c, mt=mt):
                    nc.tensor.matmul(po[:], vs_t[:, c * 128:(c + 1) * 128],
                                     mt[:], start=(c == 0),
                                     stop=(c == NCHUNK - 1))
                vq.append(vmm)
            for v in vq:
                v()
            ot = otp.tile([128, GPX], f32, tag="ot")
            nc.vector.tensor_copy(ot[:], po[:])
            nc.sync.dma_start(out[b, :, s * GPX:(s + 1) * GPX], ot[:])

        for g in range(NGRP):
            if g > 0:
                phase_b_group(g - 1)
            phase_a_tiles(g)
        phase_b_group(NGRP - 1)

        if loop_cm is not None:
            loop_cm.__exit__(None, None, None)

    nc.compile()
    return nc


def _get_program():
    if "nc" not in _CACHE:
        _CACHE["nc"] = _build_program()
    return _CACHE["nc"]


def _im2col(x):
    """x: (B,3,64,64) f32 -> cols (B, 75, 4096) f32, k=(c,dy,dx), px=(h,w)."""
    xpad = np.pad(x, ((0, 0), (0, 0), (2, 2), (2, 2)))
    win = np.lib.stride_tricks.sliding_window_view(xpad, (KH, KW), axis=(2, 3))
    cols = np.ascontiguousarray(win.transpose(0, 1, 4, 5, 2, 3))
    return cols.reshape(x.shape[0], KDIM, H * W)


def _host_prep(x, patches, values):
    """Returns per-core in_maps list."""
    pf = patches.reshape(N, KDIM)
    bias = (-0.5 * np.sum(pf.astype(np.float64) ** 2, axis=1)).astype(np.float32)

    pfull = np.zeros((KROWS, N), np.float32)
    pfull[0:KDIM] = pf.T
    pfull[KDIM] = bias
    ph = pfull.astype(np.float16)
    pl = (pfull - ph.astype(np.float32)).astype(np.float16)

    vs16 = np.ascontiguousarray(
        (values * 0.125).reshape(NCHUNK, 128, 128).transpose(1, 0, 2).reshape(128, N)
    ).astype(np.float16)
    id16 = np.eye(128, dtype=np.float16)

    cols = _im2col(x)  # (32, 75, 4096) f32
    in_maps = []
    for i in range(NCORES):
        xfull = np.empty((KROWS, PX), np.float32)
        xfull[0:KDIM] = np.concatenate(
            [cols[i * BLOC + j] for j in range(BLOC)], axis=1)
        xfull[KDIM] = 1.0
        xh = xfull.astype(np.float16)
        xl = (xfull - xh.astype(np.float32)).astype(np.float16)
        in_maps.append({"xph": xh, "xpl": xl, "ph": ph, "pl": pl,
                        "vs16": vs16, "id16": id16})
    return in_maps


def kernel(x, patches, values):
    from concourse.bass_utils import run_bass_kernel_spmd

    x = np.asarray(x, dtype=np.float32)
    patches = np.asarray(patches, dtype=np.float32)
    values = np.asarray(values, dtype=np.float32)

    nc = _get_program()
    in_maps = _host_prep(x, patches, values)
    res = run_bass_kernel_spmd(nc, in_maps, list(range(NCORES)))

    out = np.empty((B, D, H, W), np.float32)
    for i in range(NCORES):
        o = res.results[i]["out"]  # (BLOC, 128, 4096)
        out[i * BLOC:(i + 1) * BLOC] = o.reshape(BLOC, D, H, W)
    return out


# revision 25
# speedup vs baseline: 3.2740x; 3.2740x over previous
"""Trainium2 Bass kernel for NeighborsValuesAssigner (retrieval_knn).

out[b,:,h,w] = mean_{n in top8} values[n]  where top8 = 8 largest
score[b,n,h,w] = <p_n, x_patch(b,h,w)> - 0.5||p_n||^2  (5x5 'same' conv).

8 cores, data-parallel over batch (4 images/core). Per core, per
128-pixel tile:
  score  score[px, n] on PE as 3 accumulating fp16 matmuls (hi/lo fp16
         split: xh@ph + xh@pl + xl@ph; error ~2^-22, below fp32
         rounding) -> PSUM [128,1024] halves.
  drain  ACT copies each PSUM half to SBUF f32 (bit-exact); DVE max8
         per half + merge -> t8 = 8th-largest score per pixel.
  mask   DVE is_ge(score_sbuf, t8) -> {0,1} fp16 mask [px, n] (exact:
         same-arithmetic inclusive compare).
Per 512-pixel group (4 tiles), software-pipelined one group behind:
  maskT  PE transposes (fp16; these run at full 2.4 GHz while matmuls
         are power-throttled to 1.2 GHz on this part) -> PSUM -> ACT
         drain to SBUF.
  matmul out[D,px] += (values/8)[n,D]^T @ maskT[n,px] over 16 n-chunks
         (fp16 operands, f32 PSUM accumulation); DVE copy -> DMA out.

Engine budget per 512-px unit: PE ~31us (12 mms/tile + 64 transposes +
16 value mms), ACT ~17us (PSUM drains), DVE ~20us (max8/is_ge/out).
"""
import sys

sys.path.insert(0, "/opt/trn_rl_repo")

import numpy as np

B, C, H, W = 32, 3, 64, 64
N, D = 2048, 128
KH = KW = 5
KDIM = C * KH * KW          # 75
KROWS = KDIM + 1            # 76 = patch dims + bias row
NCORES = 8
BLOC = B // NCORES          # 4 images per core
PX = BLOC * H * W           # 16384 pixels per core
IPX = H * W                 # 4096 pixels per image
NTILE = PX // 128           # 128 pixel-tiles per core
GPX = 512                   # pixels per group
NGRP = PX // GPX            # 32 groups per core
NCHUNK = N // 128           # 16 patch chunks

_CACHE = {}


def _build_program(loop_r=0):
    """loop_r=0: straight-line. loop_r>0: wrap body in a device-side
    For_i loop running it loop_r times (for HW timing via wall deltas)."""
    import concourse.bacc as bacc
    import concourse.tile as tile
    import concourse.mybir as mybir
    from contextlib import ExitStack

    f32 = mybir.dt.float32
    f16 = mybir.dt.float16
    nc = bacc.Bacc("TRN2", target_bir_lowering=False, debug=False)

    xph = nc.dram_tensor("xph", [KROWS, PX], f16, kind="ExternalInput").ap()
    xpl = nc.dram_tensor("xpl", [KROWS, PX], f16, kind="ExternalInput").ap()
    phd = nc.dram_tensor("ph", [128, N], f16, kind="ExternalInput").ap()
    pld = nc.dram_tensor("pl", [128, N], f16, kind="ExternalInput").ap()
    vsd = nc.dram_tensor("vs16", [128, N], f16, kind="ExternalInput").ap()
    idd = nc.dram_tensor("id16", [128, 128], f16, kind="ExternalInput").ap()
    out = nc.dram_tensor("out", [BLOC, 128, H * W], f32, kind="ExternalOutput").ap()

    isge = mybir.AluOpType.is_ge

    with tile.TileContext(nc) as tc, ExitStack() as ctx:
        const = ctx.enter_context(tc.tile_pool(name="const", bufs=1))
        xbig = ctx.enter_context(tc.tile_pool(name="xbig", bufs=1))
        scp = ctx.enter_context(tc.tile_pool(name="scp", bufs=2))
        m16p = ctx.enter_context(tc.tile_pool(name="m16p", bufs=2))
        m8p = ctx.enter_context(tc.tile_pool(name="m8p", bufs=2))
        mkp = ctx.enter_context(tc.tile_pool(name="mkp", bufs=12))
        mtp = ctx.enter_context(tc.tile_pool(name="mtp", bufs=5))
        otp = ctx.enter_context(tc.tile_pool(name="otp", bufs=2))
        psA = ctx.enter_context(tc.tile_pool(name="psA", bufs=2, space="PSUM"))
        pst = ctx.enter_context(tc.tile_pool(name="pst", bufs=2, space="PSUM"))
        psB = ctx.enter_context(tc.tile_pool(name="psB", bufs=2, space="PSUM"))

        ph_t = const.tile([128, N], f16)
        pl_t = const.tile([128, N], f16)
        vs_t = const.tile([128, N], f16)
        id_t = const.tile([128, 128], f16)
        nc.sync.dma_start(ph_t[:], phd[:])
        nc.sync.dma_start(pl_t[:], pld[:])
        nc.sync.dma_start(vs_t[:], vsd[:])
        nc.sync.dma_start(id_t[:], idd[:])

        # Contraction padded to 128 rows: matmuls with K <= ~77 issue at
        # half rate on this part (427ns vs 216ns per 512 cols). Rows
        # KROWS..127 are zeroed once and contribute exact +0.0.
        xh_im = [xbig.tile([128, IPX], f16, tag=f"xh{j}", name=f"xh{j}")
                 for j in range(BLOC)]
        xl_im = [xbig.tile([128, IPX], f16, tag=f"xl{j}", name=f"xl{j}")
                 for j in range(BLOC)]
        for j in range(BLOC):
            nc.vector.memset(xh_im[j][64:128, :], 0.0)
            nc.vector.memset(xl_im[j][64:128, :], 0.0)

        loop_cm = tc.For_i(0, loop_r, 1) if loop_r else None
        if loop_cm is not None:
            loop_cm.__enter__()

        for j in range(BLOC):
            nc.sync.dma_start(xh_im[j][0:KROWS, :], xph[:, j * IPX:(j + 1) * IPX])
            nc.sync.dma_start(xl_im[j][0:KROWS, :], xpl[:, j * IPX:(j + 1) * IPX])

        group_masks = {}

        def phase_a_tiles(g):
            """Score + threshold + mask for the 4 pixel-tiles of group g."""
            masks = []
            for tt in range(4):
                t = 4 * g + tt
                j, toff = divmod(t, IPX // 128)
                tsl = slice(toff * 128, (toff + 1) * 128)
                lh = xh_im[j][:, tsl]
                ll = xl_im[j][:, tsl]
                sc = scp.tile([128, N], f32, tag="sc")
                m16 = m16p.tile([128, 16], f32, tag="m16")
                m8 = m8p.tile([128, 8], f32, tag="m8")
                mk = mkp.tile([128, N], f16, tag="mk")
                for h in range(2):
                    pa = psA.tile([128, 1024], f32, tag="pa")
                    for q in range(2):
                        nsl = slice(h * 1024 + q * 512, h * 1024 + (q + 1) * 512)
                        osl = slice(q * 512, (q + 1) * 512)
                        nc.tensor.matmul(pa[:, osl], lh, ph_t[:, nsl],
                                         start=True, stop=False)
                    for q in range(2):
                        nsl = slice(h * 1024 + q * 512, h * 1024 + (q + 1) * 512)
                        osl = slice(q * 512, (q + 1) * 512)
                        nc.tensor.matmul(pa[:, osl], lh, pl_t[:, nsl],
                                         start=False, stop=False)
                    for q in range(2):
                        nsl = slice(h * 1024 + q * 512, h * 1024 + (q + 1) * 512)
                        osl = slice(q * 512, (q + 1) * 512)
                        nc.tensor.matmul(pa[:, osl], ll, ph_t[:, nsl],
                                         start=False, stop=True)
                    nc.scalar.copy(sc[:, h * 1024:(h + 1) * 1024], pa[:])
                    nc.vector.max(m16[:, h * 8:(h + 1) * 8], pa[:])
                nc.vector.max(m8[:], m16[:])
                nc.vector.tensor_scalar(mk[:], sc[:], m8[:, 7:8], None, isge)
                masks.append(mk)
            group_masks[g] = masks

        def phase_b_group(g):
            """Transpose masks + values matmul for group g. The value
            matmuls trail the transposes by 3 chunks so the PSUM->SBUF
            drain (split 12 ACT / 4 DVE) never stalls the PE."""
            b, s = divmod(g, IPX // GPX)
            masks = group_masks.pop(g)
            po = psB.tile([128, GPX], f32, tag="po")
            vq = []
            for c in range(NCHUNK):
                csl = slice(c * 128, (c + 1) * 128)
                pt = pst.tile([128, GPX], f16, tag="pt")
                for tt in range(4):
                    nc.tensor.transpose(pt[:, tt * 128:(tt + 1) * 128],
                                        masks[tt][:, csl], id_t[:])
                mt = mtp.tile([128, GPX], f16, tag="mt")
                if c % 2 == 0 and c > 0:
                    nc.vector.tensor_copy(mt[:], pt[:])
                else:
                    nc.scalar.copy(mt[:], pt[:])

                def vmm(c=c, mt=mt):
                    nc.tensor.matmul(po[:], vs_t[:, c * 128:(c + 1) * 128],
                                     mt[:], start=(c == 0),
                                     stop=(c == NCHUNK - 1))
                vq.append(vmm)
                while len(vq) > 3:
                    vq.pop(0)()
            for v in vq:
                v()
            ot = otp.tile([128, GPX], f32, tag="ot")
            nc.vector.tensor_copy(ot[:], po[:])
            nc.sync.dma_start(out[b, :, s * GPX:(s + 1) * GPX], ot[:])

        for g in range(NGRP):
            if g > 0:
                phase_b_group(g - 1)
            phase_a_tiles(g)
        phase_b_group(NGRP - 1)

        if loop_cm is not None:
            loop_cm.__exit__(None, None, None)

    nc.compile()
    return nc


def _get_program():
    if "nc" not in _CACHE:
        _CACHE["nc"] = _build_program()
    return _CACHE["nc"]


def _im2col(x):
    """x: (B,3,64,64) f32 -> cols (B, 75, 4096) f32, k=(c,dy,dx), px=(h,w)."""
    xpad = np.pad(x, ((0, 0), (0, 0), (2, 2), (2, 2)))
    win = np.lib.stride_tricks.sliding_window_view(xpad, (KH, KW), axis=(2, 3))
    cols = np.ascontiguousarray(win.transpose(0, 1, 4, 5, 2, 3))
    return cols.reshape(x.shape[0], KDIM, H * W)


def _host_prep(x, patches, values):
    """Returns per-core in_maps list."""
    pf = patches.reshape(N, KDIM)
    bias = (-0.5 * np.sum(pf.astype(np.float64) ** 2, axis=1)).astype(np.float32)

    pfull = np.zeros((128, N), np.float32)
    pfull[0:KDIM] = pf.T
    pfull[KDIM] = bias
    ph = pfull.astype(np.float16)
    pl = (pfull - ph.astype(np.float32)).astype(np.float16)

    vs16 = np.ascontiguousarray(
        (values * 0.125).reshape(NCHUNK, 128, 128).transpose(1, 0, 2).reshape(128, N)
    ).astype(np.float16)
    id16 = np.eye(128, dtype=np.float16)

    cols = _im2col(x)  # (32, 75, 4096) f32
    in_maps = []
    for i in range(NCORES):
        xfull = np.empty((KROWS, PX), np.float32)
        xfull[0:KDIM] = np.concatenate(
            [cols[i * BLOC + j] for j in range(BLOC)], axis=1)
        xfull[KDIM] = 1.0
        xh = xfull.astype(np.float16)
        xl = (xfull - xh.astype(np.float32)).astype(np.float16)
        in_maps.append({"xph": xh, "xpl": xl, "ph": ph, "pl": pl,
                        "vs16": vs16, "id16": id16})
    return in_maps


def kernel(x, patches, values):
    from concourse.bass_utils import run_bass_kernel_spmd

    x = np.asarray(x, dtype=np.float32)
    patches = np.asarray(patches, dtype=np.float32)
    values = np.asarray(values, dtype=np.float32)

    nc = _get_program()
    in_maps = _host_prep(x, patches, values)
    res = run_bass_kernel_spmd(nc, in_maps, list(range(NCORES)))

    out = np.empty((B, D, H, W), np.float32)
    for i in range(NCORES):
        o = res.results[i]["out"]  # (BLOC, 128, 4096)
        out[i * BLOC:(i + 1) * BLOC] = o.reshape(BLOC, D, H, W)
    return out


# revision 40
# speedup vs baseline: 3.8153x; 1.1653x over previous
"""Trainium2 Bass kernel for NeighborsValuesAssigner (retrieval_knn).

out[b,:,h,w] = mean_{n in top8} values[n]  where top8 = 8 largest
score[b,n,h,w] = <p_n, x_patch(b,h,w)> - 0.5||p_n||^2  (5x5 'same' conv).

8 cores, data-parallel over batch (4 images/core). Per core, per
128-pixel tile:
  score  score[px, n] on PE as 3 accumulating fp16 matmuls (hi/lo fp16
         split: xh@ph + xh@pl + xl@ph; error ~2^-22, below fp32
         rounding) -> PSUM [128,1024] halves.
  drain  ACT copies each PSUM half to SBUF f32 (bit-exact); DVE max8
         per half + merge -> t8 = 8th-largest score per pixel.
  mask   DVE is_ge(score_sbuf, t8) -> {0,1} fp16 mask [px, n] (exact:
         same-arithmetic inclusive compare).
Per 512-pixel group (4 tiles), software-pipelined one group behind:
  maskT  PE transposes (fp16; these run at full 2.4 GHz while matmuls
         are power-throttled to 1.2 GHz on this part) -> PSUM -> ACT
         drain to SBUF.
  matmul out[D,px] += (values/8)[n,D]^T @ maskT[n,px] over 16 n-chunks
         (fp16 operands, f32 PSUM accumulation); DVE copy -> DMA out.

Engine budget per 512-px unit: PE ~31us (12 mms/tile + 64 transposes +
16 value mms), ACT ~17us (PSUM drains), DVE ~20us (max8/is_ge/out).
"""
import sys

sys.path.insert(0, "/opt/trn_rl_repo")

import numpy as np

B, C, H, W = 32, 3, 64, 64
N, D = 2048, 128
KH = KW = 5
KDIM = C * KH * KW          # 75
KROWS = KDIM + 1            # 76 = patch dims + bias row
NCORES = 8
BLOC = B // NCORES          # 4 images per core
PX = BLOC * H * W           # 16384 pixels per core
IPX = H * W                 # 4096 pixels per image
NTILE = PX // 128           # 128 pixel-tiles per core
GPX = 512                   # pixels per group
NGRP = PX // GPX            # 32 groups per core
NCHUNK = N // 128           # 16 patch chunks

_CACHE = {}


def _build_program(loop_r=0):
    """loop_r=0: straight-line. loop_r>0: wrap body in a device-side
    For_i loop running it loop_r times (for HW timing via wall deltas)."""
    import concourse.bacc as bacc
    import concourse.tile as tile
    import concourse.mybir as mybir
    from contextlib import ExitStack

    f32 = mybir.dt.float32
    f16 = mybir.dt.float16
    nc = bacc.Bacc("TRN2", target_bir_lowering=False, debug=False)

    xph = nc.dram_tensor("xph", [KROWS, PX], f16, kind="ExternalInput").ap()
    xpl = nc.dram_tensor("xpl", [KROWS, PX], f16, kind="ExternalInput").ap()
    phd = nc.dram_tensor("ph", [128, N], f16, kind="ExternalInput").ap()
    pld = nc.dram_tensor("pl", [128, N], f16, kind="ExternalInput").ap()
    vsd = nc.dram_tensor("vs16", [128, N], f16, kind="ExternalInput").ap()
    idd = nc.dram_tensor("id16", [128, 128], f16, kind="ExternalInput").ap()
    svd = nc.dram_tensor("sv96", [96, 128], f16, kind="ExternalInput").ap()
    oned = nc.dram_tensor("ones96", [96, GPX], f16, kind="ExternalInput").ap()
    out = nc.dram_tensor("out", [BLOC, 128, H * W], f32, kind="ExternalOutput").ap()

    isge = mybir.AluOpType.is_ge
    SIGN = mybir.ActivationFunctionType.Sign
    EPS = 1.0e-4

    with tile.TileContext(nc) as tc, ExitStack() as ctx:
        const = ctx.enter_context(tc.tile_pool(name="const", bufs=1))
        xbig = ctx.enter_context(tc.tile_pool(name="xbig", bufs=1))
        scp = ctx.enter_context(tc.tile_pool(name="scp", bufs=2))
        m16p = ctx.enter_context(tc.tile_pool(name="m16p", bufs=2))
        m8p = ctx.enter_context(tc.tile_pool(name="m8p", bufs=2))
        mkp = ctx.enter_context(tc.tile_pool(name="mkp", bufs=12))
        mtp = ctx.enter_context(tc.tile_pool(name="mtp", bufs=5))
        otp = ctx.enter_context(tc.tile_pool(name="otp", bufs=2))
        psA = ctx.enter_context(tc.tile_pool(name="psA", bufs=2, space="PSUM"))
        pst = ctx.enter_context(tc.tile_pool(name="pst", bufs=2, space="PSUM"))
        psB = ctx.enter_context(tc.tile_pool(name="psB", bufs=2, space="PSUM"))

        ph_t = const.tile([128, N], f16)
        pl_t = const.tile([128, N], f16)
        vs_t = const.tile([128, N], f16)
        id_t = const.tile([128, 128], f16)
        sv_t = const.tile([96, 128], f16)
        on_t = const.tile([96, GPX], f16)
        nc.sync.dma_start(ph_t[:], phd[:])
        nc.sync.dma_start(pl_t[:], pld[:])
        nc.sync.dma_start(vs_t[:], vsd[:])
        nc.sync.dma_start(id_t[:], idd[:])
        nc.sync.dma_start(sv_t[:], svd[:])
        nc.sync.dma_start(on_t[:], oned[:])

        # Contraction padded to 128 rows: matmuls with K <= ~77 issue at
        # half rate on this part (427ns vs 216ns per 512 cols). Rows
        # KROWS..127 are zeroed once and contribute exact +0.0.
        xh_im = [xbig.tile([128, IPX], f16, tag=f"xh{j}", name=f"xh{j}")
                 for j in range(BLOC)]
        xl_im = [xbig.tile([128, IPX], f16, tag=f"xl{j}", name=f"xl{j}")
                 for j in range(BLOC)]
        for j in range(BLOC):
            nc.vector.memset(xh_im[j][64:128, :], 0.0)
            nc.vector.memset(xl_im[j][64:128, :], 0.0)

        loop_cm = tc.For_i(0, loop_r, 1) if loop_r else None
        if loop_cm is not None:
            loop_cm.__enter__()

        for j in range(BLOC):
            nc.sync.dma_start(xh_im[j][0:KROWS, :], xph[:, j * IPX:(j + 1) * IPX])
            nc.sync.dma_start(xl_im[j][0:KROWS, :], xpl[:, j * IPX:(j + 1) * IPX])

        group_masks = {}
        sign_q = []

        def phase_a_tiles(g):
            """Score + threshold + mask for the 4 pixel-tiles of group g.

            Masks are +-1 on both halves: ACT Sign(t8 - eps - score) = -1
            iff selected (the eps margin keeps the threshold element
            itself selected despite Sign(0) == 0; extra-include rate
            ~eps/gap ~ 2e-4). Sign emission lags one tile so ACT's FIFO
            never waits on the DVE max8 -> b8 chain."""
            masks = []
            for tt in range(4):
                t = 4 * g + tt
                j, toff = divmod(t, IPX // 128)
                tsl = slice(toff * 128, (toff + 1) * 128)
                lh = xh_im[j][:, tsl]
                ll = xl_im[j][:, tsl]
                sc = scp.tile([128, N], f32, tag="sc")
                m8 = m8p.tile([128, 8], f32, tag="m8")
                b8 = m8p.tile([128, 1], f32, tag="b8")
                mk = mkp.tile([128, N], f16, tag="mk")
                for h in range(2):
                    pa = psA.tile([128, 1024], f32, tag="pa")
                    for q in range(2):
                        nsl = slice(h * 1024 + q * 512, h * 1024 + (q + 1) * 512)
                        osl = slice(q * 512, (q + 1) * 512)
                        nc.tensor.matmul(pa[:, osl], lh, ph_t[:, nsl],
                                         start=True, stop=False)
                    for q in range(2):
                        nsl = slice(h * 1024 + q * 512, h * 1024 + (q + 1) * 512)
                        osl = slice(q * 512, (q + 1) * 512)
                        nc.tensor.matmul(pa[:, osl], lh, pl_t[:, nsl],
                                         start=False, stop=False)
                    for q in range(2):
                        nsl = slice(h * 1024 + q * 512, h * 1024 + (q + 1) * 512)
                        osl = slice(q * 512, (q + 1) * 512)
                        nc.tensor.matmul(pa[:, osl], ll, ph_t[:, nsl],
                                         start=False, stop=True)
                    # drain to SBUF immediately (frees the PSUM slot fast)
                    nc.scalar.copy(sc[:, h * 1024:(h + 1) * 1024], pa[:])
                nc.vector.max(m8[:], sc[:])
                # half 0: exact inclusive is_ge -> {0,1} mask on DVE
                nc.vector.tensor_scalar(mk[:, 0:1024], sc[:, 0:1024],
                                        m8[:, 7:8], None, isge)
                nc.vector.tensor_scalar(b8[:], m8[:, 7:8], EPS, None,
                                        mybir.AluOpType.subtract)
                if sign_q:
                    sign_q.pop(0)()

                def sgn(mk=mk, sc=sc, b8=b8):
                    nc.scalar.activation(mk[:, 1024:2048], sc[:, 1024:2048],
                                         SIGN, bias=b8[:], scale=-1.0)
                sign_q.append(sgn)
                masks.append(mk)
            while sign_q:
                sign_q.pop(0)()
            group_masks[g] = masks

        def phase_b_group(g):
            """Transpose masks + values matmul for group g. The value
            matmuls trail the transposes by 3 chunks so the PSUM->SBUF
            drain (split 12 ACT / 4 DVE) never stalls the PE."""
            b, s = divmod(g, IPX // GPX)
            masks = group_masks.pop(g)
            po = psB.tile([128, GPX], f32, tag="po")
            # correction term for the +-1 masks of half 1:
            # out += sum_{n>=1024} values[n]/16 (hi/lo fp16 rows)
            nc.tensor.matmul(po[:], sv_t[:], on_t[:], start=True, stop=False)
            vq = []
            for c in range(NCHUNK):
                csl = slice(c * 128, (c + 1) * 128)
                pt = pst.tile([128, GPX], f16, tag="pt")
                for tt in range(4):
                    nc.tensor.transpose(pt[:, tt * 128:(tt + 1) * 128],
                                        masks[tt][:, csl], id_t[:])
                mt = mtp.tile([128, GPX], f16, tag="mt")
                if c % 3 == 0 and c < 15:
                    nc.scalar.copy(mt[:], pt[:])
                else:
                    nc.vector.tensor_copy(mt[:], pt[:])

                def vmm(c=c, mt=mt):
                    nc.tensor.matmul(po[:], vs_t[:, c * 128:(c + 1) * 128],
                                     mt[:], start=False,
                                     stop=(c == NCHUNK - 1))
                vq.append(vmm)
                while len(vq) > 3:
                    vq.pop(0)()
            for v in vq:
                v()
            ot = otp.tile([128, GPX], f32, tag="ot")
            nc.vector.tensor_copy(ot[:], po[:])
            nc.sync.dma_start(out[b, :, s * GPX:(s + 1) * GPX], ot[:])

        for g in range(NGRP):
            if g > 0:
                phase_b_group(g - 1)
            phase_a_tiles(g)
        phase_b_group(NGRP - 1)

        if loop_cm is not None:
            loop_cm.__exit__(None, None, None)

    nc.compile()
    return nc


def _get_program():
    if "nc" not in _CACHE:
        _CACHE["nc"] = _build_program()
    return _CACHE["nc"]


def _im2col(x):
    """x: (B,3,64,64) f32 -> cols (B, 75, 4096) f32, k=(c,dy,dx), px=(h,w)."""
    xpad = np.pad(x, ((0, 0), (0, 0), (2, 2), (2, 2)))
    win = np.lib.stride_tricks.sliding_window_view(xpad, (KH, KW), axis=(2, 3))
    cols = np.ascontiguousarray(win.transpose(0, 1, 4, 5, 2, 3))
    return cols.reshape(x.shape[0], KDIM, H * W)


def _host_prep(x, patches, values):
    """Returns per-core in_maps list."""
    pf = patches.reshape(N, KDIM)
    bias = (-0.5 * np.sum(pf.astype(np.float64) ** 2, axis=1)).astype(np.float32)

    pfull = np.zeros((128, N), np.float32)
    pfull[0:KDIM] = pf.T
    pfull[KDIM] = bias
    ph = pfull.astype(np.float16)
    pl = (pfull - ph.astype(np.float32)).astype(np.float16)

    # Half-0 masks are {0,1}: chunks 0..7 carry +values/8. Half-1 masks
    # are +-1 Sign (-1 = selected): chunks 8..15 carry -values/16 and each
    # group adds the correction sum_{n>=1024} fp16(values[n]/16) so that
    # selected rows contribute values/8 and unselected rows cancel.
    vsc = np.empty((N, D), np.float32)
    vsc[0:N // 2] = values[0:N // 2] * 0.125
    vsc[N // 2:] = values[N // 2:] * -0.0625
    vs16 = np.ascontiguousarray(
        vsc.reshape(NCHUNK, 128, 128).transpose(1, 0, 2).reshape(128, N)
    ).astype(np.float16)
    id16 = np.eye(128, dtype=np.float16)

    svf = -vs16.astype(np.float64).reshape(128, NCHUNK, 128)[:, NCHUNK // 2:].sum(axis=(0, 1))
    svh = svf.astype(np.float16)
    svl = (svf - svh.astype(np.float64)).astype(np.float16)
    sv96 = np.zeros((96, 128), np.float16)
    sv96[0] = svh
    sv96[1] = svl
    ones96 = np.zeros((96, GPX), np.float16)
    ones96[0] = 1.0
    ones96[1] = 1.0

    cols = _im2col(x)  # (32, 75, 4096) f32
    in_maps = []
    for i in range(NCORES):
        xfull = np.empty((KROWS, PX), np.float32)
        xfull[0:KDIM] = np.concatenate(
            [cols[i * BLOC + j] for j in range(BLOC)], axis=1)
        xfull[KDIM] = 1.0
        xh = xfull.astype(np.float16)
        xl = (xfull - xh.astype(np.float32)).astype(np.float16)
        in_maps.append({"xph": xh, "xpl": xl, "ph": ph, "pl": pl,
                        "vs16": vs16, "id16": id16, "sv96": sv96,
                        "ones96": ones96})
    return in_maps


def kernel(x, patches, values):
    from concourse.bass_utils import run_bass_kernel_spmd

    x = np.asarray(x, dtype=np.float32)
    patches = np.asarray(patches, dtype=np.float32)
    values = np.asarray(values, dtype=np.float32)

    nc = _get_program()
    in_maps = _host_prep(x, patches, values)
    res = run_bass_kernel_spmd(nc, in_maps, list(range(NCORES)))

    out = np.empty((B, D, H, W), np.float32)
    for i in range(NCORES):
        o = res.results[i]["out"]  # (BLOC, 128, 4096)
        out[i * BLOC:(i + 1) * BLOC] = o.reshape(BLOC, D, H, W)
    return out
